# revision 20
# baseline (speedup 1.0000x reference)
"""AttentionBlock Trainium2 kernel (nn_AttentionBlock_74010876445388).

Strategy: data-parallel over batch (B=16 -> 2 per core x 8 cores).

The PE cost model charges matmuls by OUTPUT COLUMNS ONLY (independent of
K/M), and the PE clock p-states (0.65 -> 1.2 -> 2.4 GHz) require ~3us of
gap-free execution to reach full rate.  The v1 kernel ran attention at
~1.2 GHz because each softmax round left ~0.5us Tensor gaps (waiting on
ACT exp) and each pair boundary stalled the in-order Tensor queue ~5us
behind the reciprocal chain.  This version keeps the Tensor queue
back-to-back:

  - PSUM: S chunks are [128,512] tiles from a 3-buffer pool (3 banks),
    PV accumulators 2x[128,1024] (4 banks), and ONE shared [128,512]
    "fill" bank rotates between qkv / v^T / proj / group-stats /
    denominator-broadcast matmuls (8 banks total).
  - Each softmax round emits S(h0), a filler unit, S(h1), then the
    PREVIOUS round's PV (one-round lag) so exp latency never blocks PE.
  - Filler units (next image's qkv / v^T, previous image's proj, pair
    tails) are placed round-by-round from a static schedule, sized
    ~0.9us each to close the Tensor-vs-ACT round deficit.
  - Pair tails: reciprocal_approx_fast reads the denominator row
    directly from PSUM; both halves' 1/d rows stack into one [2,1024]
    tile and ONE K=2 matmul per 512-col chunk broadcasts them across
    partitions; the whole chain is emitted as a deferred filler inside
    the NEXT pair so the Tensor queue never waits on it.

GroupNorm via bn_stats + PE selector reduction, qkv/attention/proj all
bf16 (rel err ~7e-3 << 2e-2 gate); v^T carries a ones column that
accumulates the softmax denominator in psum row 64 for free.
"""

import numpy as np

import concourse.bass as bass
import concourse.tile as tile
from concourse import bacc, mybir
from concourse.bass_utils import run_bass_kernel_spmd

N_CORES = 8
B, C, HW_L = 16, 512, 1024  # full batch, channels, flattened spatial
BPC = B // N_CORES  # batches per core = 2
NH = 8  # heads
CH = C // NH  # 64 channels/head
NG = 32  # groups
GS = C // NG  # 16 channels/group
L = HW_L
EPS = 1e-5
F32 = mybir.dt.float32
F32R = mybir.dt.float32r
BF16 = mybir.dt.bfloat16
AF = mybir.ActivationFunctionType
OP = mybir.AluOpType

_nc_cache = None


def _build(debug=False):
    nc = bacc.Bacc("TRN2", target_bir_lowering=False)

    x2 = nc.dram_tensor("x2", [BPC, C, L], BF16, kind="ExternalInput")
    wqkT = nc.dram_tensor("wqkT", [C, 3 * C], BF16, kind="ExternalInput")
    wpT = nc.dram_tensor("wpT", [C, C], BF16, kind="ExternalInput")
    # packed per-partition constants: cols = bq[0:4] bk[4:8] beff[8:12]
    # nw[12:16] nb[16:20]
    cvec = nc.dram_tensor("cvec", [128, 20], F32, kind="ExternalInput")
    sel = nc.dram_tensor("sel", [128, 4 * NG], F32, kind="ExternalInput")
    esel = nc.dram_tensor("esel", [NG, 4 * 128], F32, kind="ExternalInput")
    out_d = nc.dram_tensor("out", [BPC, C, L], F32, kind="ExternalOutput")

    from contextlib import ExitStack

    with tile.TileContext(nc) as tc, ExitStack() as es:
        cst_pool = es.enter_context(tc.tile_pool(name="const", bufs=1))
        wstage = es.enter_context(tc.tile_pool(name="wstage", bufs=1))
        xb_pool = es.enter_context(tc.tile_pool(name="xb", bufs=2))
        h_pool = es.enter_context(tc.tile_pool(name="hb", bufs=2))
        qk_pool = es.enter_context(tc.tile_pool(name="qk", bufs=6))
        vt_pool = es.enter_context(tc.tile_pool(name="vt", bufs=2))
        e_pool = es.enter_context(tc.tile_pool(name="ee", bufs=12))
        a_pool = es.enter_context(tc.tile_pool(name="ab", bufs=8))
        au_pool = es.enter_context(tc.tile_pool(name="au", bufs=2))
        rc_pool = es.enter_context(tc.tile_pool(name="rc", bufs=2))
        sm_pool = es.enter_context(tc.tile_pool(name="sm", bufs=2))
        o_pool = es.enter_context(tc.tile_pool(name="ob", bufs=3))
        ps_s = es.enter_context(tc.tile_pool(name="ps_s", bufs=3, space="PSUM"))
        ps_a = es.enter_context(tc.tile_pool(name="ps_a", bufs=2, space="PSUM"))
        ps_f = es.enter_context(tc.tile_pool(name="ps_f", bufs=1, space="PSUM"))

        # ---- per-image state ----
        xt = {}  # b -> [4 x tiles]

        def x_dma(b):
            # split tiles across both DMA queues: a single queue takes
            # ~12us for 1MB and the first groupnorm stats gate everything
            xt[b] = []
            for j in range(4):
                x_t = xb_pool.tile([128, L], BF16, tag=f"x{j}", name=f"x{b}{j}")
                q = nc.gpsimd if j % 2 == 0 else nc.sync
                q.dma_start(x_t, x2[b, 128 * j : 128 * (j + 1), :])
                xt[b].append(x_t)

        # x first (needed earliest: stats gate everything), then consts
        # and weights on the sync queue
        x_dma(0)
        x_dma(1)
        st_sel = wstage.tile([128, 4 * NG], F32, tag="selst")
        nc.sync.dma_start(st_sel, sel.ap())
        sel_sb = cst_pool.tile([128, 4 * NG], F32R, tag="sel")
        nc.vector.tensor_copy(sel_sb, st_sel)
        st_esel = wstage.tile([NG, 4 * 128], F32, tag="eselst")
        nc.sync.dma_start(st_esel, esel.ap())
        esel_sb = cst_pool.tile([NG, 4 * 128], F32R, tag="esel")
        nc.vector.tensor_copy(esel_sb, st_esel)
        cv = cst_pool.tile([128, 20], F32, tag="cvec")
        nc.sync.dma_start(cv, cvec.ap())
        wq_sb = []
        for j in range(4):
            wt = cst_pool.tile([128, 3 * C], BF16, tag=f"wq{j}")
            nc.sync.dma_start(wt, wqkT[128 * j : 128 * (j + 1), :])
            wq_sb.append(wt)
        wp_sb = []
        for j in range(4):
            wt = cst_pool.tile([128, C], BF16, tag=f"wp{j}")
            nc.sync.dma_start(wt, wpT[128 * j : 128 * (j + 1), :])
            wp_sb.append(wt)
        bq_sb = [cv[:, i : i + 1] for i in range(0, 4)]
        bk_sb = [cv[:, i : i + 1] for i in range(4, 8)]
        beff_sb = [cv[:, i : i + 1] for i in range(8, 12)]
        nw_sb = [cv[:, i : i + 1] for i in range(12, 16)]
        nb_sb = [cv[:, i : i + 1] for i in range(16, 20)]
        ones_b = cst_pool.tile([128, 1], BF16, tag="ones_b")
        nc.vector.memset(ones_b, 1.0)
        ones1b = cst_pool.tile([1, 64], BF16, tag="ones1b")
        nc.vector.memset(ones1b, 1.0)

        # contiguous slices into the host-reordered weight:
        # cols [0:512]=q pair-major, [512:1024]=k pair-major,
        # [1024:1536]=v head-major
        def wq_ap(j, p):
            return wq_sb[j][:, 128 * p : 128 * (p + 1)]

        def wk_ap(j, p):
            return wq_sb[j][:, 512 + 128 * p : 512 + 128 * (p + 1)]

        gsb_st = {}  # b -> group-stats sbuf tile
        ht = {}  # b -> [4 h tiles]
        vt_st = {}  # b -> vt_sb
        qk_st = {}  # (b, p) -> [q_sb or None, k_sb or None]
        a_st = {}  # (b, p) -> normalized a tile
        pend = {}  # (b, p) -> (a_un, rcb) awaiting tail
        proj_hold = {}  # (b, m, n) -> open psum tile between half-units

        def stats_unit(b):
            """bn stats for all 4 x tiles + PE group reduce -> gsb."""
            gps = ps_f.tile([128, 512], F32, tag="fill")
            for j in range(4):
                stats = sm_pool.tile([128, 2, 6], F32, tag="bnst")
                nc.vector.bn_stats(stats[:, 0, :], xt[b][j][:, 0:512])
                nc.vector.bn_stats(stats[:, 1, :], xt[b][j][:, 512:1024])
                mv = sm_pool.tile([128, 2], F32, tag=f"mv{j}")
                nc.vector.bn_aggr(mv, stats)
                ss = sm_pool.tile([128, 2], F32R, tag=f"ss{j}")
                msq = sm_pool.tile([128, 1], F32, tag=f"msq{j}")
                nc.vector.tensor_copy(ss[:, 0:1], mv[:, 0:1])
                nc.vector.tensor_tensor(msq, mv[:, 0:1], mv[:, 0:1], OP.mult)
                nc.vector.tensor_tensor(ss[:, 1:2], mv[:, 1:2], msq, OP.add)
                nc.tensor.matmul(
                    gps[0:NG, 0:2],
                    sel_sb[:, NG * j : NG * (j + 1)],
                    ss,
                    start=(j == 0),
                    stop=(j == 3),
                )
            gsb = sm_pool.tile([NG, 2], F32, tag="gsb")
            nc.vector.tensor_copy(gsb, gps[0:NG, 0:2])
            gsb_st[b] = gsb

        def gchain_unit(b):
            """group stats -> rstd -> per-channel scale/bias -> h (bf16)."""
            gsb = gsb_st[b]
            gm2 = sm_pool.tile([NG, 1], F32, tag="gm2")
            nc.vector.tensor_tensor(gm2, gsb[:, 0:1], gsb[:, 0:1], OP.mult)
            gvar = sm_pool.tile([NG, 1], F32, tag="gvar")
            nc.vector.tensor_tensor(gvar, gsb[:, 1:2], gm2, OP.subtract)
            nc.vector.tensor_scalar_add(gvar, gvar, EPS)
            # rsqrt via bit trick + 2 Newton iterations (no ACT table swap)
            y_i = sm_pool.tile([NG, 1], mybir.dt.int32, tag="rs_i")
            nc.vector.tensor_scalar(
                y_i, gvar.bitcast(mybir.dt.int32), 1, None, OP.logical_shift_right
            )
            nc.vector.tensor_scalar(y_i, y_i, -1, 0x5F3759DF, OP.mult, OP.add)
            y = y_i.bitcast(F32)
            gstats = sm_pool.tile([NG, 2], F32R, tag="gst")
            nc.vector.tensor_copy(gstats[:, 0:1], gsb[:, 0:1])
            tmp = sm_pool.tile([NG, 1], F32, tag="rs_t")
            for _ in range(2):
                nc.vector.tensor_tensor(tmp, y, y, OP.mult)
                nc.vector.tensor_tensor(tmp, tmp, gvar, OP.mult)
                nc.vector.tensor_scalar(tmp, tmp, -0.5, 1.5, OP.mult, OP.add)
                nc.vector.tensor_tensor(y, y, tmp, OP.mult)
            nc.vector.tensor_copy(gstats[:, 1:2], y)

            ht[b] = []
            for j in range(4):
                cst_ps = ps_f.tile([128, 512], F32, tag="fill")
                nc.tensor.matmul(
                    cst_ps[:, 0:2],
                    esel_sb[:, 128 * j : 128 * (j + 1)],
                    gstats,
                    start=True,
                    stop=True,
                )
                sc = sm_pool.tile([128, 1], F32, tag=f"sc{j}")
                nc.vector.tensor_tensor(sc, cst_ps[:, 1:2], nw_sb[j], OP.mult)
                bi = sm_pool.tile([128, 1], F32, tag=f"bi{j}")
                nc.vector.tensor_tensor(bi, cst_ps[:, 0:1], sc, OP.mult)
                nc.vector.tensor_tensor(bi, nb_sb[j], bi, OP.subtract)
                h_t = h_pool.tile([128, L], BF16, tag=f"h{j}")
                nc.vector.tensor_scalar(h_t, xt[b][j], sc, bi, OP.mult, OP.add)
                ht[b].append(h_t)

        def vt_init(b):
            """allocate vt and fill the ones (denominator) column."""
            vt_sb = vt_pool.tile([128, 8, 8, 65], BF16, tag="vt")
            nc.vector.tensor_copy(
                vt_sb[:, :, :, 64:65],
                ones_b[:, None, None, :].to_broadcast((128, 8, 8, 1)),
            )
            vt_st[b] = vt_sb

        def vt_unit(b, i):
            """v^T rows 128i..128(i+1) for all 8 heads (+ones col kept)."""
            vps = ps_f.tile([128, 512], F32, tag="fill")
            for j in range(4):
                nc.tensor.matmul(
                    vps,
                    ht[b][j][:, 128 * i : 128 * (i + 1)],
                    wq_sb[j][:, 1024:1536],
                    start=(j == 0),
                    stop=(j == 3),
                )
            nc.vector.tensor_copy(
                vt_st[b][:, i, :, 0:64],
                vps.rearrange("p (h c) -> p h c", c=64),
            )

        def qk_unit(b, p, which, n):
            """q (which=0) or k (which=1) projection for pair p, col half n."""
            ps = ps_f.tile([128, 512], F32, tag="fill")
            w_ap = wq_ap if which == 0 else wk_ap
            for j in range(4):
                nc.tensor.matmul(
                    ps,
                    w_ap(j, p),
                    ht[b][j][:, 512 * n : 512 * (n + 1)],
                    start=(j == 0),
                    stop=(j == 3),
                )
            if (b, p) not in qk_st:
                qk_st[(b, p)] = [None, None]
            if qk_st[(b, p)][which] is None:
                tag = "q" if which == 0 else "k"
                qk_st[(b, p)][which] = qk_pool.tile(
                    [128, L], BF16, tag=tag, name=f"{tag}{b}{p}"
                )
            sb = qk_st[(b, p)][which]
            bias = (bq_sb if which == 0 else bk_sb)[p]
            nc.vector.tensor_scalar_add(sb[:, 512 * n : 512 * (n + 1)], ps, bias)

        def tail_unit(b, p):
            """denominator broadcast + normalize for a finished pair."""
            a_un, rcb = pend.pop((b, p))
            a_t = a_pool.tile([128, L], BF16, tag="a")
            for n in range(2):
                nsl = slice(512 * n, 512 * (n + 1))
                bc = ps_f.tile([128, 512], F32, tag="fill")
                nc.tensor.matmul(
                    bc[0:64, :], ones1b, rcb[0][:, nsl],
                    start=True, stop=True, tile_position=(0, 0),
                )
                nc.tensor.matmul(
                    bc[64:128, :], ones1b, rcb[1][:, nsl],
                    start=True, stop=True, tile_position=(0, 64),
                )
                nc.vector.tensor_tensor(a_t[:, nsl], a_un[:, nsl], bc, OP.mult)
            a_st[(b, p)] = a_t

        def _proj_mm(ps, b, m, n, js, start, stop):
            nsl = slice(512 * n, 512 * (n + 1))
            for j in js:
                nc.tensor.matmul(
                    ps,
                    wp_sb[j][:, 128 * m : 128 * (m + 1)],
                    a_st[(b, j)][:, nsl],
                    start=(start and j == js[0]),
                    stop=(stop and j == js[-1]),
                )

        def _proj_out(ps, b, m, n):
            nsl = slice(512 * n, 512 * (n + 1))
            o_t = o_pool.tile([128, 512], F32, tag="o")
            nc.vector.tensor_scalar_add(o_t, ps, beff_sb[m])
            nc.sync.dma_start(out_d[b, 128 * m : 128 * (m + 1), nsl], o_t)

        def proj_a(b, m, n):
            """first half of a projection chunk (j=0,1); psum stays open."""
            ps = ps_f.tile([128, 512], F32, tag="fill", name=f"pjA{b}{m}{n}")
            _proj_mm(ps, b, m, n, (0, 1), True, False)
            proj_hold[(b, m, n)] = ps

        def proj_b(b, m, n):
            """second half (j=2,3) + bias + store. MUST be the next ps_f
            user after the matching proj_a (open accumulation group)."""
            ps = proj_hold.pop((b, m, n))
            _proj_mm(ps, b, m, n, (2, 3), False, True)
            _proj_out(ps, b, m, n)

        def proj_unit(b, m, n, pool=None, tag="fill"):
            """full projection chunk in one unit (for the drain tail)."""
            pool = pool or ps_f
            ps = pool.tile([128, 512], F32, tag=tag, name=f"pj{b}{m}{n}")
            _proj_mm(ps, b, m, n, (0, 1, 2, 3), True, True)
            _proj_out(ps, b, m, n)

        def emit_pair(b, p, fills):
            """8 softmax rounds; PV lags S by one round; fills[r] emitted
            between S(h0) and S(h1) of round r."""
            q_sb, k_sb = qk_st.pop((b, p))
            vt_sb = vt_st[b]
            a_ps = [
                ps_a.tile([128, 1024], F32, tag="apv", name=f"apvA{b}{p}"),
                ps_a.tile([128, 1024], F32, tag="apv", name=f"apvB{b}{p}"),
            ]

            def emit_pv(sj, e_r):
                for h in range(2):
                    for n in range(2):
                        nsl = slice(512 * n, 512 * (n + 1))
                        nc.tensor.matmul(
                            a_ps[h][0:65, nsl],
                            vt_sb[:, sj, 2 * p + h, :],
                            e_r[h][n],
                            start=(sj == 0),
                            stop=(sj == 7),
                        )

            e_hist = []
            for sj in range(8):
                cur_e = [[None, None], [None, None]]
                for h in range(2):
                    for n in range(2):
                        st = ps_s.tile([128, 512], F32, tag="s")
                        nc.tensor.matmul(
                            st,
                            k_sb[64 * h : 64 * (h + 1), 128 * sj : 128 * (sj + 1)],
                            q_sb[64 * h : 64 * (h + 1), 512 * n : 512 * (n + 1)],
                            start=True,
                            stop=True,
                            tile_position=(64 * h, 0),
                        )
                        et = e_pool.tile([128, 512], BF16, tag="e")
                        nc.scalar.activation(et, st, AF.Exp, scale=0.125)
                        cur_e[h][n] = et
                    if h == 0:
                        for f in fills[sj]:
                            f()
                e_hist.append(cur_e)
                # PV lags S by TWO rounds so the in-order Tensor queue
                # never waits on the ACT exp pipeline (the round limiter)
                if sj >= 2:
                    emit_pv(sj - 2, e_hist[sj - 2])
            emit_pv(6, e_hist[6])
            emit_pv(7, e_hist[7])

            # evacuate PV + denominator reciprocals (frees psum quickly;
            # the broadcast+normalize runs later as a tail_unit filler).
            # On the very last pair ACT is idle, so splitting the chain
            # between ACT and DVE halves the serial latency; mid-kernel
            # ACT is exp-saturated, so keep everything on DVE there.
            final = b == BPC - 1 and p == 3
            a_un = au_pool.tile([128, L], BF16, tag="a_un")
            dsb = [
                rc_pool.tile([1, 1024], F32, tag="dsb0", name="dsb0"),
                rc_pool.tile([1, 1024], F32, tag="dsb1", name="dsb1"),
            ]
            if final:
                nc.scalar.activation(a_un[0:64, :], a_ps[0][0:64, :], AF.Copy)
                nc.scalar.activation(dsb[0], a_ps[0][64:65, :], AF.Copy)
            else:
                nc.vector.tensor_copy(a_un[0:64, :], a_ps[0][0:64, :])
                nc.vector.tensor_copy(dsb[0], a_ps[0][64:65, :])
            nc.vector.tensor_copy(a_un[64:128, :], a_ps[1][0:64, :])
            nc.vector.tensor_copy(dsb[1], a_ps[1][64:65, :])
            rcb = []
            for h in range(2):
                rcf = rc_pool.tile([1, 1024], F32, tag=f"rcf{h}", name=f"rcf{h}")
                nc.vector.reciprocal_approx_fast(rcf, dsb[h])
                rcbh = rc_pool.tile([1, 1024], BF16, tag=f"rcb{h}", name=f"rcb{h}")
                if final:
                    nc.vector.tensor_copy(rcbh, rcf)
                else:
                    # sbuf->sbuf cast on the idle GpSimd engine keeps the
                    # DVE queue clear for fill evacuations (GpSimd cannot
                    # read PSUM, so the psum-side copies stay on DVE)
                    nc.gpsimd.tensor_copy(rcbh, rcf)
                rcb.append(rcbh)
            pend[(b, p)] = (a_un, rcb)

        # ================= schedule =================
        NOP = []

        def F(*fns):
            return list(fns)

        def mk(fn, *args):
            return lambda: fn(*args)

        # ---- startup (image 0) ----
        stats_unit(0)
        gchain_unit(0)
        vt_init(0)
        for p in (0, 1):
            for which in (0, 1):
                for n in (0, 1):
                    qk_unit(0, p, which, n)

        # filler schedule per (image, pair): 8 slots, each a list.
        # Roughly one ~0.9us tensor unit per round so the PE never idles
        # (idle resets the clock p-state to half rate).  proj_a/proj_b
        # halves MUST occupy consecutive ps_f slots (open psum group).
        # tails sit at round 2 so the pair's reciprocal chain (DVE/GpSimd)
        # has ~2 rounds of cover before the bc matmuls need its result
        sched = {}
        sched[(0, 0)] = [F(mk(vt_unit, 0, i)) for i in range(8)]
        sched[(0, 1)] = [
            F(mk(qk_unit, 0, 2, 0, 0)),
            F(mk(qk_unit, 0, 2, 0, 1)),
            F(mk(tail_unit, 0, 0)),
            F(mk(qk_unit, 0, 2, 1, 0)),
            F(mk(qk_unit, 0, 2, 1, 1)),
            F(mk(stats_unit, 1), mk(qk_unit, 0, 3, 0, 0)),
            F(mk(qk_unit, 0, 3, 0, 1)),
            F(mk(qk_unit, 0, 3, 1, 0)),
        ]
        sched[(0, 2)] = [
            F(mk(qk_unit, 0, 3, 1, 1)),
            F(mk(gchain_unit, 1)),
            F(mk(tail_unit, 0, 1)),
            F(mk(vt_init, 1), mk(qk_unit, 1, 0, 0, 0)),
            F(mk(qk_unit, 1, 0, 0, 1)),
            F(mk(qk_unit, 1, 0, 1, 0)),
            F(mk(qk_unit, 1, 0, 1, 1)),
            F(mk(qk_unit, 1, 1, 0, 0)),
        ]
        sched[(0, 3)] = [
            F(mk(qk_unit, 1, 1, 0, 1)),
            F(mk(qk_unit, 1, 1, 1, 0)),
            F(mk(tail_unit, 0, 2)),
            F(mk(qk_unit, 1, 1, 1, 1)),
            F(mk(vt_unit, 1, 0)),
            F(mk(vt_unit, 1, 1)),
            F(mk(vt_unit, 1, 2)),
            F(mk(vt_unit, 1, 3)),
        ]
        sched[(1, 0)] = [
            F(mk(vt_unit, 1, 4)),
            F(mk(vt_unit, 1, 5)),
            F(mk(tail_unit, 0, 3)),
            F(mk(vt_unit, 1, 6)),
            F(mk(vt_unit, 1, 7)),
            F(mk(qk_unit, 1, 2, 0, 0)),
            F(mk(qk_unit, 1, 2, 0, 1)),
            F(mk(qk_unit, 1, 2, 1, 0)),
        ]
        sched[(1, 1)] = [
            F(mk(qk_unit, 1, 2, 1, 1)),
            F(mk(qk_unit, 1, 3, 0, 0)),
            F(mk(tail_unit, 1, 0)),
            F(mk(qk_unit, 1, 3, 0, 1)),
            F(mk(qk_unit, 1, 3, 1, 0)),
            F(mk(qk_unit, 1, 3, 1, 1)),
            F(mk(proj_a, 0, 0, 0)),
            F(mk(proj_b, 0, 0, 0)),
        ]
        sched[(1, 2)] = [
            F(mk(proj_a, 0, 0, 1)),
            F(mk(proj_b, 0, 0, 1)),
            F(mk(tail_unit, 1, 1)),
            F(mk(proj_a, 0, 1, 0)),
            F(mk(proj_b, 0, 1, 0)),
            NOP,
            F(mk(proj_a, 0, 1, 1)),
            F(mk(proj_b, 0, 1, 1)),
        ]
        sched[(1, 3)] = [
            F(mk(proj_a, 0, 2, 0)),
            F(mk(proj_b, 0, 2, 0)),
            F(mk(tail_unit, 1, 2)),
            F(mk(proj_a, 0, 2, 1)),
            F(mk(proj_b, 0, 2, 1)),
            F(mk(proj_a, 0, 3, 0)),
            F(mk(proj_b, 0, 3, 0)),
            F(mk(proj_unit, 0, 3, 1)),
        ]

        for b in range(BPC):
            for p in range(4):
                emit_pair(b, p, sched[(b, p)])

        # drain tail: last pair's normalize + image-1 projection,
        # alternating psum pools so the units double-buffer
        tail_unit(1, 3)
        for i, (m, n) in enumerate((m, n) for m in range(4) for n in range(2)):
            if i % 2 == 0:
                proj_unit(1, m, n)
            else:
                proj_unit(1, m, n, pool=ps_s, tag="s")

    nc.compile()
    return nc


def _get_nc():
    global _nc_cache
    if _nc_cache is None:
        _nc_cache = _build()
    return _nc_cache


def _prep_inputs(x, norm_w, norm_b, w_qkv, b_qkv, w_proj, b_proj):
    import ml_dtypes

    x = np.asarray(x, dtype=np.float32).reshape(B, C, L)
    w_qkv = np.asarray(w_qkv, dtype=np.float32)
    b_qkv = np.asarray(b_qkv, dtype=np.float32)
    w_proj = np.asarray(w_proj, dtype=np.float32)
    b_proj = np.asarray(b_proj, dtype=np.float32)
    norm_w = np.asarray(norm_w, dtype=np.float32)
    norm_b = np.asarray(norm_b, dtype=np.float32)

    # column-reordered transposed qkv weight: [C, 3C] with
    # q pair-major | k pair-major | v head-major, all contiguous
    wqkT = np.zeros((C, 3 * C), dtype=np.float32)
    wT = w_qkv.T  # [C, 3C] original row order (per head: q,k,v)
    for h in range(NH):
        base = 192 * h
        wqkT[:, 64 * h : 64 * (h + 1)] = wT[:, base : base + 64]
        wqkT[:, 512 + 64 * h : 512 + 64 * (h + 1)] = wT[:, base + 64 : base + 128]
        wqkT[:, 1024 + 64 * h : 1024 + 64 * (h + 1)] = wT[:, base + 128 : base + 192]
    wqkT = np.ascontiguousarray(wqkT.astype(ml_dtypes.bfloat16))
    wpT = np.ascontiguousarray(w_proj.T.astype(ml_dtypes.bfloat16))  # [C, C]

    # per-pair q/k biases: [pair, {q,k}, 128]
    bqk = np.zeros((4, 2, 128), dtype=np.float32)
    for p in range(4):
        for half, h in enumerate((2 * p, 2 * p + 1)):
            base = 192 * h
            bqk[p, 0, 64 * half : 64 * (half + 1)] = b_qkv[base : base + 64]
            bqk[p, 1, 64 * half : 64 * (half + 1)] = b_qkv[base + 64 : base + 128]
    del base

    # v bias folded into proj bias: b_eff = b_proj + w_proj @ bv
    bv = np.zeros((C,), dtype=np.float32)
    for h in range(NH):
        bv[64 * h : 64 * (h + 1)] = b_qkv[192 * h + 128 : 192 * h + 192]
    b_eff = (b_proj.astype(np.float64) + w_proj.astype(np.float64) @ bv).astype(
        np.float32
    )

    sel = np.zeros((128, 4 * NG), dtype=np.float32)
    esel = np.zeros((NG, 4 * 128), dtype=np.float32)
    for j in range(4):
        for c in range(128):
            sel[c, NG * j + 8 * j + c // GS] = 1.0 / GS
            esel[8 * j + c // GS, 128 * j + c] = 1.0

    cv = np.zeros((128, 20), dtype=np.float32)
    cv[:, 0:4] = bqk[:, 0, :].T
    cv[:, 4:8] = bqk[:, 1, :].T
    cv[:, 8:12] = b_eff.reshape(4, 128).T
    cv[:, 12:16] = norm_w.reshape(4, 128).T
    cv[:, 16:20] = norm_b.reshape(4, 128).T

    shared = {
        "wqkT": wqkT,
        "wpT": wpT,
        "cvec": cv,
        "sel": sel,
        "esel": esel,
    }
    in_maps = []
    for c in range(N_CORES):
        m = dict(shared)
        m["x2"] = np.ascontiguousarray(
            x[BPC * c : BPC * (c + 1)].astype(ml_dtypes.bfloat16)
        )
        in_maps.append(m)
    return in_maps


def _run(in_maps, trace=False):
    nc = _get_nc()
    return run_bass_kernel_spmd(
        nc, in_maps, core_ids=list(range(N_CORES)), trace=trace
    )


def kernel(x, norm_w, norm_b, w_qkv, b_qkv, w_proj, b_proj):
    in_maps = _prep_inputs(x, norm_w, norm_b, w_qkv, b_qkv, w_proj, b_proj)
    res = _run(in_maps)
    out = np.concatenate([r["out"] for r in res.results], axis=0)
    return out.astype(np.float32)


# revision 21
# speedup vs baseline: 1.2255x; 1.2255x over previous
"""AttentionBlock Trainium2 kernel (nn_AttentionBlock_74010876445388).

Strategy: data-parallel over batch (B=16 -> 2 per core x 8 cores).

The PE cost model charges matmuls by OUTPUT COLUMNS ONLY (independent of
K/M), and the PE clock p-states (0.65 -> 1.2 -> 2.4 GHz) require ~3us of
gap-free execution to reach full rate.  The v1 kernel ran attention at
~1.2 GHz because each softmax round left ~0.5us Tensor gaps (waiting on
ACT exp) and each pair boundary stalled the in-order Tensor queue ~5us
behind the reciprocal chain.  This version keeps the Tensor queue
back-to-back:

  - PSUM: S chunks are [128,512] tiles from a 3-buffer pool (3 banks),
    PV accumulators 2x[128,1024] (4 banks), and ONE shared [128,512]
    "fill" bank rotates between qkv / v^T / proj / group-stats /
    denominator-broadcast matmuls (8 banks total).
  - Each softmax round emits S(h0), a filler unit, S(h1), then the
    PREVIOUS round's PV (one-round lag) so exp latency never blocks PE.
  - Filler units (next image's qkv / v^T, previous image's proj, pair
    tails) are placed round-by-round from a static schedule, sized
    ~0.9us each to close the Tensor-vs-ACT round deficit.
  - Pair tails: reciprocal_approx_fast reads the denominator row
    directly from PSUM; both halves' 1/d rows stack into one [2,1024]
    tile and ONE K=2 matmul per 512-col chunk broadcasts them across
    partitions; the whole chain is emitted as a deferred filler inside
    the NEXT pair so the Tensor queue never waits on it.

GroupNorm via bn_stats + PE selector reduction, qkv/attention/proj all
bf16 (rel err ~7e-3 << 2e-2 gate); v^T carries a ones column that
accumulates the softmax denominator in psum row 64 for free.
"""

import numpy as np

import concourse.bass as bass
import concourse.tile as tile
from concourse import bacc, mybir
from concourse.bass_utils import run_bass_kernel_spmd

N_CORES = 8
B, C, HW_L = 16, 512, 1024  # full batch, channels, flattened spatial
BPC = B // N_CORES  # batches per core = 2
NH = 8  # heads
CH = C // NH  # 64 channels/head
NG = 32  # groups
GS = C // NG  # 16 channels/group
L = HW_L
EPS = 1e-5
F32 = mybir.dt.float32
F32R = mybir.dt.float32r
BF16 = mybir.dt.bfloat16
AF = mybir.ActivationFunctionType
OP = mybir.AluOpType

_nc_cache = None


def _build(debug=False):
    nc = bacc.Bacc("TRN2", target_bir_lowering=False)

    x2 = nc.dram_tensor("x2", [BPC, C, L], BF16, kind="ExternalInput")
    wqkT = nc.dram_tensor("wqkT", [C, 3 * C], BF16, kind="ExternalInput")
    wpT = nc.dram_tensor("wpT", [C, C], BF16, kind="ExternalInput")
    # packed per-partition constants: cols = bq[0:4] bk[4:8] beff[8:12]
    # nw[12:16] nb[16:20]
    cvec = nc.dram_tensor("cvec", [128, 20], F32, kind="ExternalInput")
    sel = nc.dram_tensor("sel", [128, 4 * NG], F32, kind="ExternalInput")
    esel = nc.dram_tensor("esel", [NG, 4 * 128], F32, kind="ExternalInput")
    out_d = nc.dram_tensor("out", [BPC, C, L], F32, kind="ExternalOutput")

    from contextlib import ExitStack

    with tile.TileContext(nc) as tc, ExitStack() as es:
        cst_pool = es.enter_context(tc.tile_pool(name="const", bufs=1))
        wstage = es.enter_context(tc.tile_pool(name="wstage", bufs=1))
        xb_pool = es.enter_context(tc.tile_pool(name="xb", bufs=2))
        h_pool = es.enter_context(tc.tile_pool(name="hb", bufs=2))
        qk_pool = es.enter_context(tc.tile_pool(name="qk", bufs=6))
        vt_pool = es.enter_context(tc.tile_pool(name="vt", bufs=2))
        e_pool = es.enter_context(tc.tile_pool(name="ee", bufs=12))
        a_pool = es.enter_context(tc.tile_pool(name="ab", bufs=8))
        au_pool = es.enter_context(tc.tile_pool(name="au", bufs=2))
        rc_pool = es.enter_context(tc.tile_pool(name="rc", bufs=2))
        sm_pool = es.enter_context(tc.tile_pool(name="sm", bufs=2))
        o_pool = es.enter_context(tc.tile_pool(name="ob", bufs=3))
        ps_s = es.enter_context(tc.tile_pool(name="ps_s", bufs=3, space="PSUM"))
        ps_a = es.enter_context(tc.tile_pool(name="ps_a", bufs=2, space="PSUM"))
        ps_f = es.enter_context(tc.tile_pool(name="ps_f", bufs=1, space="PSUM"))

        # ---- per-image state ----
        xt = {}  # b -> [4 x tiles]

        def x_dma(b):
            # split tiles across both DMA queues: a single queue takes
            # ~12us for 1MB and the first groupnorm stats gate everything
            xt[b] = []
            for j in range(4):
                x_t = xb_pool.tile([128, L], BF16, tag=f"x{j}", name=f"x{b}{j}")
                q = nc.gpsimd if j % 2 == 0 else nc.sync
                q.dma_start(x_t, x2[b, 128 * j : 128 * (j + 1), :])
                xt[b].append(x_t)

        # x first (needed earliest: stats gate everything), then consts
        # and weights on the sync queue
        x_dma(0)
        x_dma(1)
        st_sel = wstage.tile([128, 4 * NG], F32, tag="selst")
        nc.sync.dma_start(st_sel, sel.ap())
        sel_sb = cst_pool.tile([128, 4 * NG], F32R, tag="sel")
        nc.vector.tensor_copy(sel_sb, st_sel)
        st_esel = wstage.tile([NG, 4 * 128], F32, tag="eselst")
        nc.sync.dma_start(st_esel, esel.ap())
        esel_sb = cst_pool.tile([NG, 4 * 128], F32R, tag="esel")
        nc.vector.tensor_copy(esel_sb, st_esel)
        cv = cst_pool.tile([128, 20], F32, tag="cvec")
        nc.sync.dma_start(cv, cvec.ap())
        wq_sb = []
        for j in range(4):
            wt = cst_pool.tile([128, 3 * C], BF16, tag=f"wq{j}")
            nc.sync.dma_start(wt, wqkT[128 * j : 128 * (j + 1), :])
            wq_sb.append(wt)
        wp_sb = []
        for j in range(4):
            wt = cst_pool.tile([128, C], BF16, tag=f"wp{j}")
            nc.sync.dma_start(wt, wpT[128 * j : 128 * (j + 1), :])
            wp_sb.append(wt)
        bq_sb = [cv[:, i : i + 1] for i in range(0, 4)]
        bk_sb = [cv[:, i : i + 1] for i in range(4, 8)]
        beff_sb = [cv[:, i : i + 1] for i in range(8, 12)]
        nw_sb = [cv[:, i : i + 1] for i in range(12, 16)]
        nb_sb = [cv[:, i : i + 1] for i in range(16, 20)]
        ones_b = cst_pool.tile([128, 1], BF16, tag="ones_b")
        nc.vector.memset(ones_b, 1.0)
        ones1b = cst_pool.tile([1, 64], BF16, tag="ones1b")
        nc.vector.memset(ones1b, 1.0)

        # contiguous slices into the host-reordered weight:
        # cols [0:512]=q pair-major, [512:1024]=k pair-major,
        # [1024:1536]=v head-major
        def wq_ap(j, p):
            return wq_sb[j][:, 128 * p : 128 * (p + 1)]

        def wk_ap(j, p):
            return wq_sb[j][:, 512 + 128 * p : 512 + 128 * (p + 1)]

        gsb_st = {}  # b -> group-stats sbuf tile
        ht = {}  # b -> [4 h tiles]
        vt_st = {}  # b -> vt_sb
        qk_st = {}  # (b, p) -> [q_sb or None, k_sb or None]
        a_st = {}  # (b, p) -> normalized a tile
        pend = {}  # (b, p) -> (a_un, rcb) awaiting tail
        proj_hold = {}  # (b, m, n) -> open psum tile between half-units

        def stats_unit(b):
            """bn stats for all 4 x tiles + PE group reduce -> gsb."""
            gps = ps_f.tile([128, 512], F32, tag="fill")
            for j in range(4):
                stats = sm_pool.tile([128, 2, 6], F32, tag="bnst")
                nc.vector.bn_stats(stats[:, 0, :], xt[b][j][:, 0:512])
                nc.vector.bn_stats(stats[:, 1, :], xt[b][j][:, 512:1024])
                mv = sm_pool.tile([128, 2], F32, tag=f"mv{j}")
                nc.vector.bn_aggr(mv, stats)
                ss = sm_pool.tile([128, 2], F32R, tag=f"ss{j}")
                msq = sm_pool.tile([128, 1], F32, tag=f"msq{j}")
                nc.vector.tensor_copy(ss[:, 0:1], mv[:, 0:1])
                nc.vector.tensor_tensor(msq, mv[:, 0:1], mv[:, 0:1], OP.mult)
                nc.vector.tensor_tensor(ss[:, 1:2], mv[:, 1:2], msq, OP.add)
                nc.tensor.matmul(
                    gps[0:NG, 0:2],
                    sel_sb[:, NG * j : NG * (j + 1)],
                    ss,
                    start=(j == 0),
                    stop=(j == 3),
                )
            gsb = sm_pool.tile([NG, 2], F32, tag="gsb")
            nc.vector.tensor_copy(gsb, gps[0:NG, 0:2])
            gsb_st[b] = gsb

        def gchain_unit(b):
            """group stats -> rstd -> per-channel scale/bias -> h (bf16)."""
            gsb = gsb_st[b]
            gm2 = sm_pool.tile([NG, 1], F32, tag="gm2")
            nc.vector.tensor_tensor(gm2, gsb[:, 0:1], gsb[:, 0:1], OP.mult)
            gvar = sm_pool.tile([NG, 1], F32, tag="gvar")
            nc.vector.tensor_tensor(gvar, gsb[:, 1:2], gm2, OP.subtract)
            nc.vector.tensor_scalar_add(gvar, gvar, EPS)
            # rsqrt via bit trick + 2 Newton iterations (no ACT table swap)
            y_i = sm_pool.tile([NG, 1], mybir.dt.int32, tag="rs_i")
            nc.vector.tensor_scalar(
                y_i, gvar.bitcast(mybir.dt.int32), 1, None, OP.logical_shift_right
            )
            nc.vector.tensor_scalar(y_i, y_i, -1, 0x5F3759DF, OP.mult, OP.add)
            y = y_i.bitcast(F32)
            gstats = sm_pool.tile([NG, 2], F32R, tag="gst")
            nc.vector.tensor_copy(gstats[:, 0:1], gsb[:, 0:1])
            tmp = sm_pool.tile([NG, 1], F32, tag="rs_t")
            for _ in range(2):
                nc.vector.tensor_tensor(tmp, y, y, OP.mult)
                nc.vector.tensor_tensor(tmp, tmp, gvar, OP.mult)
                nc.vector.tensor_scalar(tmp, tmp, -0.5, 1.5, OP.mult, OP.add)
                nc.vector.tensor_tensor(y, y, tmp, OP.mult)
            nc.vector.tensor_copy(gstats[:, 1:2], y)

            ht[b] = []
            for j in range(4):
                cst_ps = ps_f.tile([128, 512], F32, tag="fill")
                nc.tensor.matmul(
                    cst_ps[:, 0:2],
                    esel_sb[:, 128 * j : 128 * (j + 1)],
                    gstats,
                    start=True,
                    stop=True,
                )
                sc = sm_pool.tile([128, 1], F32, tag=f"sc{j}")
                nc.vector.tensor_tensor(sc, cst_ps[:, 1:2], nw_sb[j], OP.mult)
                bi = sm_pool.tile([128, 1], F32, tag=f"bi{j}")
                nc.vector.tensor_tensor(bi, cst_ps[:, 0:1], sc, OP.mult)
                nc.vector.tensor_tensor(bi, nb_sb[j], bi, OP.subtract)
                h_t = h_pool.tile([128, L], BF16, tag=f"h{j}")
                nc.vector.tensor_scalar(h_t, xt[b][j], sc, bi, OP.mult, OP.add)
                ht[b].append(h_t)

        def vt_init(b):
            """allocate vt and fill the ones (denominator) column."""
            vt_sb = vt_pool.tile([128, 8, 8, 65], BF16, tag="vt")
            nc.vector.tensor_copy(
                vt_sb[:, :, :, 64:65],
                ones_b[:, None, None, :].to_broadcast((128, 8, 8, 1)),
            )
            vt_st[b] = vt_sb

        def vt_unit(b, i):
            """v^T rows 128i..128(i+1) for all 8 heads (+ones col kept)."""
            vps = ps_f.tile([128, 512], F32, tag="fill")
            for j in range(4):
                nc.tensor.matmul(
                    vps,
                    ht[b][j][:, 128 * i : 128 * (i + 1)],
                    wq_sb[j][:, 1024:1536],
                    start=(j == 0),
                    stop=(j == 3),
                )
            nc.vector.tensor_copy(
                vt_st[b][:, i, :, 0:64],
                vps.rearrange("p (h c) -> p h c", c=64),
            )

        def qk_unit(b, p, which, n):
            """q (which=0) or k (which=1) projection for pair p, col half n."""
            ps = ps_f.tile([128, 512], F32, tag="fill")
            w_ap = wq_ap if which == 0 else wk_ap
            for j in range(4):
                nc.tensor.matmul(
                    ps,
                    w_ap(j, p),
                    ht[b][j][:, 512 * n : 512 * (n + 1)],
                    start=(j == 0),
                    stop=(j == 3),
                )
            if (b, p) not in qk_st:
                qk_st[(b, p)] = [None, None]
            if qk_st[(b, p)][which] is None:
                tag = "q" if which == 0 else "k"
                qk_st[(b, p)][which] = qk_pool.tile(
                    [128, L], BF16, tag=tag, name=f"{tag}{b}{p}"
                )
            sb = qk_st[(b, p)][which]
            bias = (bq_sb if which == 0 else bk_sb)[p]
            nc.vector.tensor_scalar_add(sb[:, 512 * n : 512 * (n + 1)], ps, bias)

        def tail_unit(b, p):
            """denominator broadcast + normalize for a finished pair."""
            a_un, rcb = pend.pop((b, p))
            a_t = a_pool.tile([128, L], BF16, tag="a")
            for n in range(2):
                nsl = slice(512 * n, 512 * (n + 1))
                bc = ps_f.tile([128, 512], F32, tag="fill")
                nc.tensor.matmul(
                    bc[0:64, :], ones1b, rcb[0][:, nsl],
                    start=True, stop=True, tile_position=(0, 0),
                )
                nc.tensor.matmul(
                    bc[64:128, :], ones1b, rcb[1][:, nsl],
                    start=True, stop=True, tile_position=(0, 64),
                )
                nc.vector.tensor_tensor(a_t[:, nsl], a_un[:, nsl], bc, OP.mult)
            a_st[(b, p)] = a_t

        def _proj_mm(ps, b, m, n, js, start, stop):
            nsl = slice(512 * n, 512 * (n + 1))
            for j in js:
                nc.tensor.matmul(
                    ps,
                    wp_sb[j][:, 128 * m : 128 * (m + 1)],
                    a_st[(b, j)][:, nsl],
                    start=(start and j == js[0]),
                    stop=(stop and j == js[-1]),
                )

        def _proj_out(ps, b, m, n):
            nsl = slice(512 * n, 512 * (n + 1))
            o_t = o_pool.tile([128, 512], F32, tag="o")
            nc.vector.tensor_scalar_add(o_t, ps, beff_sb[m])
            nc.sync.dma_start(out_d[b, 128 * m : 128 * (m + 1), nsl], o_t)

        def proj_a(b, m, n):
            """first half of a projection chunk (j=0,1); psum stays open."""
            ps = ps_f.tile([128, 512], F32, tag="fill", name=f"pjA{b}{m}{n}")
            _proj_mm(ps, b, m, n, (0, 1), True, False)
            proj_hold[(b, m, n)] = ps

        def proj_b(b, m, n):
            """second half (j=2,3) + bias + store. MUST be the next ps_f
            user after the matching proj_a (open accumulation group)."""
            ps = proj_hold.pop((b, m, n))
            _proj_mm(ps, b, m, n, (2, 3), False, True)
            _proj_out(ps, b, m, n)

        def proj_unit(b, m, n, pool=None, tag="fill"):
            """full projection chunk in one unit (for the drain tail)."""
            pool = pool or ps_f
            ps = pool.tile([128, 512], F32, tag=tag, name=f"pj{b}{m}{n}")
            _proj_mm(ps, b, m, n, (0, 1, 2, 3), True, True)
            _proj_out(ps, b, m, n)

        def emit_pair(b, p, fills):
            """8 softmax rounds; PV lags S by one round; fills[r] emitted
            between S(h0) and S(h1) of round r."""
            q_sb, k_sb = qk_st.pop((b, p))
            vt_sb = vt_st[b]
            a_ps = [
                ps_a.tile([128, 1024], F32, tag="apv", name=f"apvA{b}{p}"),
                ps_a.tile([128, 1024], F32, tag="apv", name=f"apvB{b}{p}"),
            ]

            def emit_pv(sj, e_r):
                for h in range(2):
                    for n in range(2):
                        nsl = slice(512 * n, 512 * (n + 1))
                        nc.tensor.matmul(
                            a_ps[h][0:65, nsl],
                            vt_sb[:, sj, 2 * p + h, :],
                            e_r[h][n],
                            start=(sj == 0),
                            stop=(sj == 7),
                        )

            e_hist = []
            for sj in range(8):
                cur_e = [[None, None], [None, None]]
                for h in range(2):
                    for n in range(2):
                        st = ps_s.tile([128, 512], F32, tag="s")
                        nc.tensor.matmul(
                            st,
                            k_sb[64 * h : 64 * (h + 1), 128 * sj : 128 * (sj + 1)],
                            q_sb[64 * h : 64 * (h + 1), 512 * n : 512 * (n + 1)],
                            start=True,
                            stop=True,
                            tile_position=(64 * h, 0),
                        )
                        et = e_pool.tile([128, 512], BF16, tag="e")
                        nc.scalar.activation(et, st, AF.Exp, scale=0.125)
                        cur_e[h][n] = et
                    if h == 0:
                        for f in fills[sj]:
                            f()
                e_hist.append(cur_e)
                # PV lags S by TWO rounds so the in-order Tensor queue
                # never waits on the ACT exp pipeline (the round limiter)
                if sj >= 2:
                    emit_pv(sj - 2, e_hist[sj - 2])
            emit_pv(6, e_hist[6])
            emit_pv(7, e_hist[7])

            # evacuate PV + denominator reciprocals (frees psum quickly;
            # the broadcast+normalize runs later as a tail_unit filler).
            # On the very last pair ACT is idle, so splitting the chain
            # between ACT and DVE halves the serial latency; mid-kernel
            # ACT is exp-saturated, so keep everything on DVE there.
            final = b == BPC - 1 and p == 3
            a_un = au_pool.tile([128, L], BF16, tag="a_un")
            dsb = [
                rc_pool.tile([1, 1024], F32, tag="dsb0", name="dsb0"),
                rc_pool.tile([1, 1024], F32, tag="dsb1", name="dsb1"),
            ]
            if final:
                nc.scalar.activation(a_un[0:64, :], a_ps[0][0:64, :], AF.Copy)
                nc.scalar.activation(dsb[0], a_ps[0][64:65, :], AF.Copy)
            else:
                nc.vector.tensor_copy(a_un[0:64, :], a_ps[0][0:64, :])
                nc.vector.tensor_copy(dsb[0], a_ps[0][64:65, :])
            nc.vector.tensor_copy(a_un[64:128, :], a_ps[1][0:64, :])
            nc.vector.tensor_copy(dsb[1], a_ps[1][64:65, :])
            rcb = []
            for h in range(2):
                rcf = rc_pool.tile([1, 1024], F32, tag=f"rcf{h}", name=f"rcf{h}")
                nc.vector.reciprocal_approx_fast(rcf, dsb[h])
                rcbh = rc_pool.tile([1, 1024], BF16, tag=f"rcb{h}", name=f"rcb{h}")
                nc.vector.tensor_copy(rcbh, rcf)
                rcb.append(rcbh)
            pend[(b, p)] = (a_un, rcb)

        # ================= schedule =================
        NOP = []

        def F(*fns):
            return list(fns)

        def mk(fn, *args):
            return lambda: fn(*args)

        # ---- startup (image 0) ----
        stats_unit(0)
        gchain_unit(0)
        vt_init(0)
        for p in (0, 1):
            for which in (0, 1):
                for n in (0, 1):
                    qk_unit(0, p, which, n)

        # filler schedule per (image, pair): 8 slots, each a list.
        # Roughly one ~0.9us tensor unit per round so the PE never idles
        # (idle resets the clock p-state to half rate).  proj_a/proj_b
        # halves MUST occupy consecutive ps_f slots (open psum group).
        # tails sit at round 2 so the pair's reciprocal chain (DVE/GpSimd)
        # has ~2 rounds of cover before the bc matmuls need its result
        sched = {}
        sched[(0, 0)] = [F(mk(vt_unit, 0, i)) for i in range(8)]
        sched[(0, 1)] = [
            F(mk(qk_unit, 0, 2, 0, 0)),
            F(mk(qk_unit, 0, 2, 0, 1)),
            F(mk(tail_unit, 0, 0)),
            F(mk(qk_unit, 0, 2, 1, 0)),
            F(mk(qk_unit, 0, 2, 1, 1)),
            F(mk(stats_unit, 1), mk(qk_unit, 0, 3, 0, 0)),
            F(mk(qk_unit, 0, 3, 0, 1)),
            F(mk(qk_unit, 0, 3, 1, 0)),
        ]
        sched[(0, 2)] = [
            F(mk(qk_unit, 0, 3, 1, 1)),
            F(mk(gchain_unit, 1)),
            F(mk(tail_unit, 0, 1)),
            F(mk(vt_init, 1), mk(qk_unit, 1, 0, 0, 0)),
            F(mk(qk_unit, 1, 0, 0, 1)),
            F(mk(qk_unit, 1, 0, 1, 0)),
            F(mk(qk_unit, 1, 0, 1, 1)),
            F(mk(qk_unit, 1, 1, 0, 0)),
        ]
        sched[(0, 3)] = [
            F(mk(qk_unit, 1, 1, 0, 1)),
            F(mk(qk_unit, 1, 1, 1, 0)),
            F(mk(tail_unit, 0, 2)),
            F(mk(qk_unit, 1, 1, 1, 1)),
            F(mk(vt_unit, 1, 0)),
            F(mk(vt_unit, 1, 1)),
            F(mk(vt_unit, 1, 2)),
            F(mk(vt_unit, 1, 3)),
        ]
        sched[(1, 0)] = [
            F(mk(vt_unit, 1, 4)),
            F(mk(vt_unit, 1, 5)),
            F(mk(tail_unit, 0, 3)),
            F(mk(vt_unit, 1, 6)),
            F(mk(vt_unit, 1, 7)),
            F(mk(qk_unit, 1, 2, 0, 0)),
            F(mk(qk_unit, 1, 2, 0, 1)),
            F(mk(qk_unit, 1, 2, 1, 0)),
        ]
        sched[(1, 1)] = [
            F(mk(qk_unit, 1, 2, 1, 1)),
            F(mk(qk_unit, 1, 3, 0, 0)),
            F(mk(tail_unit, 1, 0)),
            F(mk(qk_unit, 1, 3, 0, 1)),
            F(mk(qk_unit, 1, 3, 1, 0)),
            F(mk(qk_unit, 1, 3, 1, 1)),
            F(mk(proj_a, 0, 0, 0)),
            F(mk(proj_b, 0, 0, 0)),
        ]
        sched[(1, 2)] = [
            F(mk(proj_a, 0, 0, 1)),
            F(mk(proj_b, 0, 0, 1)),
            F(mk(tail_unit, 1, 1)),
            F(mk(proj_a, 0, 1, 0)),
            F(mk(proj_b, 0, 1, 0)),
            NOP,
            F(mk(proj_a, 0, 1, 1)),
            F(mk(proj_b, 0, 1, 1)),
        ]
        sched[(1, 3)] = [
            F(mk(proj_a, 0, 2, 0)),
            F(mk(proj_b, 0, 2, 0)),
            F(mk(tail_unit, 1, 2)),
            F(mk(proj_a, 0, 2, 1)),
            F(mk(proj_b, 0, 2, 1)),
            F(mk(proj_a, 0, 3, 0)),
            F(mk(proj_b, 0, 3, 0)),
            F(mk(proj_unit, 0, 3, 1)),
        ]

        for b in range(BPC):
            for p in range(4):
                emit_pair(b, p, sched[(b, p)])

        # drain tail: last pair's normalize + image-1 projection,
        # alternating psum pools so the units double-buffer
        tail_unit(1, 3)
        for i, (m, n) in enumerate((m, n) for m in range(4) for n in range(2)):
            if i % 2 == 0:
                proj_unit(1, m, n)
            else:
                proj_unit(1, m, n, pool=ps_s, tag="s")

    nc.compile()
    return nc


def _get_nc():
    global _nc_cache
    if _nc_cache is None:
        _nc_cache = _build()
    return _nc_cache


def _prep_inputs(x, norm_w, norm_b, w_qkv, b_qkv, w_proj, b_proj):
    import ml_dtypes

    x = np.asarray(x, dtype=np.float32).reshape(B, C, L)
    w_qkv = np.asarray(w_qkv, dtype=np.float32)
    b_qkv = np.asarray(b_qkv, dtype=np.float32)
    w_proj = np.asarray(w_proj, dtype=np.float32)
    b_proj = np.asarray(b_proj, dtype=np.float32)
    norm_w = np.asarray(norm_w, dtype=np.float32)
    norm_b = np.asarray(norm_b, dtype=np.float32)

    # column-reordered transposed qkv weight: [C, 3C] with
    # q pair-major | k pair-major | v head-major, all contiguous
    wqkT = np.zeros((C, 3 * C), dtype=np.float32)
    wT = w_qkv.T  # [C, 3C] original row order (per head: q,k,v)
    for h in range(NH):
        base = 192 * h
        wqkT[:, 64 * h : 64 * (h + 1)] = wT[:, base : base + 64]
        wqkT[:, 512 + 64 * h : 512 + 64 * (h + 1)] = wT[:, base + 64 : base + 128]
        wqkT[:, 1024 + 64 * h : 1024 + 64 * (h + 1)] = wT[:, base + 128 : base + 192]
    wqkT = np.ascontiguousarray(wqkT.astype(ml_dtypes.bfloat16))
    wpT = np.ascontiguousarray(w_proj.T.astype(ml_dtypes.bfloat16))  # [C, C]

    # per-pair q/k biases: [pair, {q,k}, 128]
    bqk = np.zeros((4, 2, 128), dtype=np.float32)
    for p in range(4):
        for half, h in enumerate((2 * p, 2 * p + 1)):
            base = 192 * h
            bqk[p, 0, 64 * half : 64 * (half + 1)] = b_qkv[base : base + 64]
            bqk[p, 1, 64 * half : 64 * (half + 1)] = b_qkv[base + 64 : base + 128]
    del base

    # v bias folded into proj bias: b_eff = b_proj + w_proj @ bv
    bv = np.zeros((C,), dtype=np.float32)
    for h in range(NH):
        bv[64 * h : 64 * (h + 1)] = b_qkv[192 * h + 128 : 192 * h + 192]
    b_eff = (b_proj.astype(np.float64) + w_proj.astype(np.float64) @ bv).astype(
        np.float32
    )

    sel = np.zeros((128, 4 * NG), dtype=np.float32)
    esel = np.zeros((NG, 4 * 128), dtype=np.float32)
    for j in range(4):
        for c in range(128):
            sel[c, NG * j + 8 * j + c // GS] = 1.0 / GS
            esel[8 * j + c // GS, 128 * j + c] = 1.0

    cv = np.zeros((128, 20), dtype=np.float32)
    cv[:, 0:4] = bqk[:, 0, :].T
    cv[:, 4:8] = bqk[:, 1, :].T
    cv[:, 8:12] = b_eff.reshape(4, 128).T
    cv[:, 12:16] = norm_w.reshape(4, 128).T
    cv[:, 16:20] = norm_b.reshape(4, 128).T

    shared = {
        "wqkT": wqkT,
        "wpT": wpT,
        "cvec": cv,
        "sel": sel,
        "esel": esel,
    }
    in_maps = []
    for c in range(N_CORES):
        m = dict(shared)
        m["x2"] = np.ascontiguousarray(
            x[BPC * c : BPC * (c + 1)].astype(ml_dtypes.bfloat16)
        )
        in_maps.append(m)
    return in_maps


def _run(in_maps, trace=False):
    nc = _get_nc()
    return run_bass_kernel_spmd(
        nc, in_maps, core_ids=list(range(N_CORES)), trace=trace
    )


def kernel(x, norm_w, norm_b, w_qkv, b_qkv, w_proj, b_proj):
    in_maps = _prep_inputs(x, norm_w, norm_b, w_qkv, b_qkv, w_proj, b_proj)
    res = _run(in_maps)
    out = np.concatenate([r["out"] for r in res.results], axis=0)
    return out.astype(np.float32)


# revision 27
# speedup vs baseline: 1.3093x; 1.0683x over previous
"""AttentionBlock Trainium2 kernel (nn_AttentionBlock_74010876445388).

Strategy: data-parallel over batch (B=16 -> 2 per core x 8 cores).

The PE cost model charges matmuls by OUTPUT COLUMNS ONLY (independent of
K/M), and the PE clock p-states (0.65 -> 1.2 -> 2.4 GHz) require ~3us of
gap-free execution to reach full rate.  The v1 kernel ran attention at
~1.2 GHz because each softmax round left ~0.5us Tensor gaps (waiting on
ACT exp) and each pair boundary stalled the in-order Tensor queue ~5us
behind the reciprocal chain.  This version keeps the Tensor queue
back-to-back:

  - PSUM: S chunks are [128,512] tiles from a 3-buffer pool (3 banks),
    PV accumulators 2x[128,1024] (4 banks), and ONE shared [128,512]
    "fill" bank rotates between qkv / v^T / proj / group-stats /
    denominator-broadcast matmuls (8 banks total).
  - Each softmax round emits S(h0), a filler unit, S(h1), then the
    PREVIOUS round's PV (one-round lag) so exp latency never blocks PE.
  - Filler units (next image's qkv / v^T, previous image's proj, pair
    tails) are placed round-by-round from a static schedule, sized
    ~0.9us each to close the Tensor-vs-ACT round deficit.
  - Pair tails: reciprocal_approx_fast reads the denominator row
    directly from PSUM; both halves' 1/d rows stack into one [2,1024]
    tile and ONE K=2 matmul per 512-col chunk broadcasts them across
    partitions; the whole chain is emitted as a deferred filler inside
    the NEXT pair so the Tensor queue never waits on it.

GroupNorm via bn_stats + PE selector reduction, qkv/attention/proj all
bf16 (rel err ~7e-3 << 2e-2 gate); v^T carries a ones column that
accumulates the softmax denominator in psum row 64 for free.
"""

import numpy as np

import concourse.bass as bass
import concourse.tile as tile
from concourse import bacc, mybir
from concourse.bass_utils import run_bass_kernel_spmd

N_CORES = 8
B, C, HW_L = 16, 512, 1024  # full batch, channels, flattened spatial
BPC = B // N_CORES  # batches per core = 2
NH = 8  # heads
CH = C // NH  # 64 channels/head
NG = 32  # groups
GS = C // NG  # 16 channels/group
L = HW_L
EPS = 1e-5
F32 = mybir.dt.float32
F32R = mybir.dt.float32r
BF16 = mybir.dt.bfloat16
AF = mybir.ActivationFunctionType
OP = mybir.AluOpType

_nc_cache = None


def _build(debug=False):
    nc = bacc.Bacc("TRN2", target_bir_lowering=False)

    x2 = nc.dram_tensor("x2", [BPC, C, L], BF16, kind="ExternalInput")
    wqkT = nc.dram_tensor("wqkT", [C, 3 * C], BF16, kind="ExternalInput")
    wpT = nc.dram_tensor("wpT", [C, C], BF16, kind="ExternalInput")
    # packed per-partition constants: cols = bq[0:4] bk[4:8] beff[8:12]
    # nw[12:16] nb[16:20]
    cvec = nc.dram_tensor("cvec", [128, 20], F32, kind="ExternalInput")
    sel = nc.dram_tensor("sel", [128, 4 * NG], F32, kind="ExternalInput")
    esel = nc.dram_tensor("esel", [NG, 4 * 128], F32, kind="ExternalInput")
    out_d = nc.dram_tensor("out", [BPC, C, L], F32, kind="ExternalOutput")

    from contextlib import ExitStack

    with tile.TileContext(nc) as tc, ExitStack() as es:
        cst_pool = es.enter_context(tc.tile_pool(name="const", bufs=1))
        wstage = es.enter_context(tc.tile_pool(name="wstage", bufs=1))
        xb_pool = es.enter_context(tc.tile_pool(name="xb", bufs=2))
        h_pool = es.enter_context(tc.tile_pool(name="hb", bufs=2))
        qk_pool = es.enter_context(tc.tile_pool(name="qk", bufs=6))
        vt_pool = es.enter_context(tc.tile_pool(name="vt", bufs=2))
        e_pool = es.enter_context(tc.tile_pool(name="ee", bufs=12))
        a_pool = es.enter_context(tc.tile_pool(name="ab", bufs=8))
        au_pool = es.enter_context(tc.tile_pool(name="au", bufs=2))
        rc_pool = es.enter_context(tc.tile_pool(name="rc", bufs=2))
        sm_pool = es.enter_context(tc.tile_pool(name="sm", bufs=2))
        o_pool = es.enter_context(tc.tile_pool(name="ob", bufs=3))
        ps_s = es.enter_context(tc.tile_pool(name="ps_s", bufs=3, space="PSUM"))
        ps_a = es.enter_context(tc.tile_pool(name="ps_a", bufs=2, space="PSUM"))
        ps_f = es.enter_context(tc.tile_pool(name="ps_f", bufs=1, space="PSUM"))

        # ---- per-image state ----
        xt = {}  # b -> [4 x tiles]

        def x_dma(b):
            # split tiles across both DMA queues: a single queue takes
            # ~12us for 1MB and the first groupnorm stats gate everything
            xt[b] = []
            for j in range(4):
                x_t = xb_pool.tile([128, L], BF16, tag=f"x{j}", name=f"x{b}{j}")
                q = nc.gpsimd if j % 2 == 0 else nc.sync
                q.dma_start(x_t, x2[b, 128 * j : 128 * (j + 1), :])
                xt[b].append(x_t)

        # x first (needed earliest: stats gate everything), then consts
        # and weights on the sync queue
        x_dma(0)
        x_dma(1)
        st_sel = wstage.tile([128, 4 * NG], F32, tag="selst")
        nc.sync.dma_start(st_sel, sel.ap())
        sel_sb = cst_pool.tile([128, 4 * NG], F32R, tag="sel")
        nc.vector.tensor_copy(sel_sb, st_sel)
        st_esel = wstage.tile([NG, 4 * 128], F32, tag="eselst")
        nc.sync.dma_start(st_esel, esel.ap())
        esel_sb = cst_pool.tile([NG, 4 * 128], F32R, tag="esel")
        nc.vector.tensor_copy(esel_sb, st_esel)
        cv = cst_pool.tile([128, 20], F32, tag="cvec")
        nc.sync.dma_start(cv, cvec.ap())
        wq_sb = []
        for j in range(4):
            wt = cst_pool.tile([128, 3 * C], BF16, tag=f"wq{j}")
            nc.sync.dma_start(wt, wqkT[128 * j : 128 * (j + 1), :])
            wq_sb.append(wt)
        wp_sb = []
        for j in range(4):
            wt = cst_pool.tile([128, C], BF16, tag=f"wp{j}")
            nc.sync.dma_start(wt, wpT[128 * j : 128 * (j + 1), :])
            wp_sb.append(wt)
        bq_sb = [cv[:, i : i + 1] for i in range(0, 4)]
        bk_sb = [cv[:, i : i + 1] for i in range(4, 8)]
        beff_sb = [cv[:, i : i + 1] for i in range(8, 12)]
        nw_sb = [cv[:, i : i + 1] for i in range(12, 16)]
        nb_sb = [cv[:, i : i + 1] for i in range(16, 20)]
        ones_b = cst_pool.tile([128, 1], BF16, tag="ones_b")
        nc.vector.memset(ones_b, 1.0)
        ones1b = cst_pool.tile([1, 64], BF16, tag="ones1b")
        nc.vector.memset(ones1b, 1.0)
        scr1 = cst_pool.tile([1, 512], BF16, tag="scr1")
        nc.vector.memset(scr1, 1.0)

        # contiguous slices into the host-reordered weight:
        # cols [0:512]=q pair-major, [512:1024]=k pair-major,
        # [1024:1536]=v head-major
        def wq_ap(j, p):
            return wq_sb[j][:, 128 * p : 128 * (p + 1)]

        def wk_ap(j, p):
            return wq_sb[j][:, 512 + 128 * p : 512 + 128 * (p + 1)]

        gsb_st = {}  # b -> group-stats sbuf tile
        ht = {}  # b -> [4 h tiles]
        vt_st = {}  # b -> vt_sb
        qk_st = {}  # (b, p) -> [q_sb or None, k_sb or None]
        a_st = {}  # (b, p) -> normalized a tile
        pend = {}  # (b, p) -> (a_un, rcb) awaiting tail
        proj_hold = {}  # (b, m, n) -> open psum tile between half-units

        def stats_unit(b):
            """bn stats for all 4 x tiles + PE group reduce -> gsb."""
            gps = ps_f.tile([128, 512], F32, tag="fill")
            for j in range(4):
                stats = sm_pool.tile([128, 2, 6], F32, tag="bnst")
                nc.vector.bn_stats(stats[:, 0, :], xt[b][j][:, 0:512])
                nc.vector.bn_stats(stats[:, 1, :], xt[b][j][:, 512:1024])
                mv = sm_pool.tile([128, 2], F32, tag=f"mv{j}")
                nc.vector.bn_aggr(mv, stats)
                ss = sm_pool.tile([128, 2], F32R, tag=f"ss{j}")
                msq = sm_pool.tile([128, 1], F32, tag=f"msq{j}")
                nc.vector.tensor_copy(ss[:, 0:1], mv[:, 0:1])
                nc.vector.tensor_tensor(msq, mv[:, 0:1], mv[:, 0:1], OP.mult)
                nc.vector.tensor_tensor(ss[:, 1:2], mv[:, 1:2], msq, OP.add)
                nc.tensor.matmul(
                    gps[0:NG, 0:2],
                    sel_sb[:, NG * j : NG * (j + 1)],
                    ss,
                    start=(j == 0),
                    stop=(j == 3),
                )
            gsb = sm_pool.tile([NG, 2], F32, tag="gsb")
            nc.vector.tensor_copy(gsb, gps[0:NG, 0:2])
            gsb_st[b] = gsb

        def gchain_unit(b):
            """group stats -> rstd -> per-channel scale/bias -> h (bf16)."""
            gsb = gsb_st[b]
            gm2 = sm_pool.tile([NG, 1], F32, tag="gm2")
            nc.vector.tensor_tensor(gm2, gsb[:, 0:1], gsb[:, 0:1], OP.mult)
            gvar = sm_pool.tile([NG, 1], F32, tag="gvar")
            nc.vector.tensor_tensor(gvar, gsb[:, 1:2], gm2, OP.subtract)
            nc.vector.tensor_scalar_add(gvar, gvar, EPS)
            # rsqrt via bit trick + 2 Newton iterations (no ACT table swap)
            y_i = sm_pool.tile([NG, 1], mybir.dt.int32, tag="rs_i")
            nc.vector.tensor_scalar(
                y_i, gvar.bitcast(mybir.dt.int32), 1, None, OP.logical_shift_right
            )
            nc.vector.tensor_scalar(y_i, y_i, -1, 0x5F3759DF, OP.mult, OP.add)
            y = y_i.bitcast(F32)
            gstats = sm_pool.tile([NG, 2], F32R, tag="gst")
            nc.vector.tensor_copy(gstats[:, 0:1], gsb[:, 0:1])
            tmp = sm_pool.tile([NG, 1], F32, tag="rs_t")
            for _ in range(2):
                nc.vector.tensor_tensor(tmp, y, y, OP.mult)
                nc.vector.tensor_tensor(tmp, tmp, gvar, OP.mult)
                nc.vector.tensor_scalar(tmp, tmp, -0.5, 1.5, OP.mult, OP.add)
                nc.vector.tensor_tensor(y, y, tmp, OP.mult)
            nc.vector.tensor_copy(gstats[:, 1:2], y)

            ht[b] = []
            for j in range(4):
                cst_ps = ps_f.tile([128, 512], F32, tag="fill")
                nc.tensor.matmul(
                    cst_ps[:, 0:2],
                    esel_sb[:, 128 * j : 128 * (j + 1)],
                    gstats,
                    start=True,
                    stop=True,
                )
                sc = sm_pool.tile([128, 1], F32, tag=f"sc{j}")
                nc.vector.tensor_tensor(sc, cst_ps[:, 1:2], nw_sb[j], OP.mult)
                bi = sm_pool.tile([128, 1], F32, tag=f"bi{j}")
                nc.vector.tensor_tensor(bi, cst_ps[:, 0:1], sc, OP.mult)
                nc.vector.tensor_tensor(bi, nb_sb[j], bi, OP.subtract)
                h_t = h_pool.tile([128, L], BF16, tag=f"h{j}")
                nc.vector.tensor_scalar(h_t, xt[b][j], sc, bi, OP.mult, OP.add)
                ht[b].append(h_t)

        def vt_init(b):
            """allocate vt and fill the ones (denominator) column."""
            vt_sb = vt_pool.tile([128, 8, 8, 65], BF16, tag="vt")
            nc.vector.tensor_copy(
                vt_sb[:, :, :, 64:65],
                ones_b[:, None, None, :].to_broadcast((128, 8, 8, 1)),
            )
            vt_st[b] = vt_sb

        def vt_unit(b, i):
            """v^T rows 128i..128(i+1) for all 8 heads (+ones col kept)."""
            vps = ps_f.tile([128, 512], F32, tag="fill")
            for j in range(4):
                nc.tensor.matmul(
                    vps,
                    ht[b][j][:, 128 * i : 128 * (i + 1)],
                    wq_sb[j][:, 1024:1536],
                    start=(j == 0),
                    stop=(j == 3),
                )
            nc.vector.tensor_copy(
                vt_st[b][:, i, :, 0:64],
                vps.rearrange("p (h c) -> p h c", c=64),
            )

        def qk_unit(b, p, which, n):
            """q (which=0) or k (which=1) projection for pair p, col half n."""
            ps = ps_f.tile([128, 512], F32, tag="fill")
            w_ap = wq_ap if which == 0 else wk_ap
            for j in range(4):
                nc.tensor.matmul(
                    ps,
                    w_ap(j, p),
                    ht[b][j][:, 512 * n : 512 * (n + 1)],
                    start=(j == 0),
                    stop=(j == 3),
                )
            if (b, p) not in qk_st:
                qk_st[(b, p)] = [None, None]
            if qk_st[(b, p)][which] is None:
                tag = "q" if which == 0 else "k"
                qk_st[(b, p)][which] = qk_pool.tile(
                    [128, L], BF16, tag=tag, name=f"{tag}{b}{p}"
                )
            sb = qk_st[(b, p)][which]
            bias = (bq_sb if which == 0 else bk_sb)[p]
            nc.vector.tensor_scalar_add(sb[:, 512 * n : 512 * (n + 1)], ps, bias)

        def dummy_unit(k=2):
            """keep-the-PE-hot matmuls into a never-read psum tile: an
            idle PE drops its clock p-state, which then doubles the cost
            of every real matmul until ~3us of continuous work rebuilds
            it.  MUST NOT be scheduled between an open proj_a/proj_b
            half-pair (would rotate their psum bank out from under them).
            """
            ps = ps_f.tile([128, 512], F32, tag="fill", name="dmy")
            for _ in range(k):
                nc.tensor.matmul(
                    ps[0:64, :], ones1b, scr1, start=True, stop=True
                )

        def tail_unit(b, p):
            """denominator broadcast + normalize for a finished pair."""
            a_un, rcb = pend.pop((b, p))
            a_t = a_pool.tile([128, L], BF16, tag="a")
            for n in range(2):
                nsl = slice(512 * n, 512 * (n + 1))
                bc = ps_f.tile([128, 512], F32, tag="fill")
                nc.tensor.matmul(
                    bc[0:64, :], ones1b, rcb[0][:, nsl],
                    start=True, stop=True, tile_position=(0, 0),
                )
                nc.tensor.matmul(
                    bc[64:128, :], ones1b, rcb[1][:, nsl],
                    start=True, stop=True, tile_position=(0, 64),
                )
                nc.vector.tensor_tensor(a_t[:, nsl], a_un[:, nsl], bc, OP.mult)
            a_st[(b, p)] = a_t

        def _proj_mm(ps, b, m, n, js, start, stop):
            nsl = slice(512 * n, 512 * (n + 1))
            for j in js:
                nc.tensor.matmul(
                    ps,
                    wp_sb[j][:, 128 * m : 128 * (m + 1)],
                    a_st[(b, j)][:, nsl],
                    start=(start and j == js[0]),
                    stop=(stop and j == js[-1]),
                )

        def _proj_out(ps, b, m, n):
            nsl = slice(512 * n, 512 * (n + 1))
            o_t = o_pool.tile([128, 512], F32, tag="o")
            nc.vector.tensor_scalar_add(o_t, ps, beff_sb[m])
            nc.sync.dma_start(out_d[b, 128 * m : 128 * (m + 1), nsl], o_t)

        def proj_a(b, m, n):
            """first half of a projection chunk (j=0,1); psum stays open."""
            ps = ps_f.tile([128, 512], F32, tag="fill", name=f"pjA{b}{m}{n}")
            _proj_mm(ps, b, m, n, (0, 1), True, False)
            proj_hold[(b, m, n)] = ps

        def proj_b(b, m, n):
            """second half (j=2,3) + bias + store. MUST be the next ps_f
            user after the matching proj_a (open accumulation group)."""
            ps = proj_hold.pop((b, m, n))
            _proj_mm(ps, b, m, n, (2, 3), False, True)
            _proj_out(ps, b, m, n)

        def proj_unit(b, m, n, pool=None, tag="fill"):
            """full projection chunk in one unit (for the drain tail)."""
            pool = pool or ps_f
            ps = pool.tile([128, 512], F32, tag=tag, name=f"pj{b}{m}{n}")
            _proj_mm(ps, b, m, n, (0, 1, 2, 3), True, True)
            _proj_out(ps, b, m, n)

        def emit_pair(b, p, fills, dchain):
            """8 softmax rounds; PV lags S by two rounds; fills[r] emitted
            between S(h0) and S(h1) of round r.  dchain holds the previous
            pair's denominator-chain closures (slow [1,1024] DVE ops),
            spread 2-per-round so they never back up the DVE queue ahead
            of the fill evacuations.  Returns this pair's own closures."""
            q_sb, k_sb = qk_st.pop((b, p))
            vt_sb = vt_st[b]
            a_ps = [
                ps_a.tile([128, 1024], F32, tag="apv", name=f"apvA{b}{p}"),
                ps_a.tile([128, 1024], F32, tag="apv", name=f"apvB{b}{p}"),
            ]

            def emit_pv(sj, e_r):
                for h in range(2):
                    for n in range(2):
                        nsl = slice(512 * n, 512 * (n + 1))
                        nc.tensor.matmul(
                            a_ps[h][0:65, nsl],
                            vt_sb[:, sj, 2 * p + h, :],
                            e_r[h][n],
                            start=(sj == 0),
                            stop=(sj == 7),
                        )

            e_hist = []
            for sj in range(8):
                cur_e = [[None, None], [None, None]]
                for h in range(2):
                    for n in range(2):
                        st = ps_s.tile([128, 512], F32, tag="s")
                        nc.tensor.matmul(
                            st,
                            k_sb[64 * h : 64 * (h + 1), 128 * sj : 128 * (sj + 1)],
                            q_sb[64 * h : 64 * (h + 1), 512 * n : 512 * (n + 1)],
                            start=True,
                            stop=True,
                            tile_position=(64 * h, 0),
                        )
                        et = e_pool.tile([128, 512], BF16, tag="e")
                        nc.scalar.activation(et, st, AF.Exp, scale=0.125)
                        cur_e[h][n] = et
                    if h == 0:
                        for f in fills[sj]:
                            f()
                        for _ in range(2):
                            if dchain:
                                dchain.pop(0)()
                e_hist.append(cur_e)
                # PV lags S by TWO rounds so the in-order Tensor queue
                # never waits on the ACT exp pipeline (the round limiter)
                if sj >= 2:
                    emit_pv(sj - 2, e_hist[sj - 2])
            emit_pv(6, e_hist[6])
            emit_pv(7, e_hist[7])

            # evacuate PV + denominator reciprocals (frees psum quickly;
            # the broadcast+normalize runs later as a tail_unit filler).
            # On the very last pair ACT is idle, so splitting the chain
            # between ACT and DVE halves the serial latency; mid-kernel
            # ACT is exp-saturated, so keep everything on DVE there.
            final = b == BPC - 1 and p == 3
            a_un = au_pool.tile([128, L], BF16, tag="a_un")
            rcb = []
            pend[(b, p)] = (a_un, rcb)
            if final:
                # last pair: ACT is idle, split the chain across engines
                # and run it inline (no following rounds to spread into)
                dsb = [
                    rc_pool.tile([1, 1024], F32, tag="dsb0", name="dsb0"),
                    rc_pool.tile([1, 1024], F32, tag="dsb1", name="dsb1"),
                ]
                nc.scalar.activation(a_un[0:64, :], a_ps[0][0:64, :], AF.Copy)
                nc.scalar.activation(dsb[0], a_ps[0][64:65, :], AF.Copy)
                nc.vector.tensor_copy(a_un[64:128, :], a_ps[1][0:64, :])
                nc.vector.tensor_copy(dsb[1], a_ps[1][64:65, :])
                for h in range(2):
                    rcf = rc_pool.tile(
                        [1, 1024], F32, tag=f"rcf{h}", name=f"rcf{h}"
                    )
                    nc.vector.reciprocal_approx_fast(rcf, dsb[h])
                    rcbh = rc_pool.tile(
                        [1, 1024], BF16, tag=f"rcb{h}", name=f"rcb{h}"
                    )
                    nc.vector.tensor_copy(rcbh, rcf)
                    rcb.append(rcbh)
                return []
            nc.vector.tensor_copy(a_un[0:64, :], a_ps[0][0:64, :])
            nc.vector.tensor_copy(a_un[64:128, :], a_ps[1][0:64, :])
            dsb = [None, None]
            rcf = [None, None]

            def cl_dsb(h):
                dsb[h] = rc_pool.tile([1, 1024], F32, tag=f"dsb{h}", name=f"dsb{h}")
                nc.vector.tensor_copy(dsb[h], a_ps[h][64:65, :])

            def cl_rcf(h):
                rcf[h] = rc_pool.tile([1, 1024], F32, tag=f"rcf{h}", name=f"rcf{h}")
                nc.vector.reciprocal_approx_fast(rcf[h], dsb[h])

            def cl_rcb(h):
                rcbh = rc_pool.tile([1, 1024], BF16, tag=f"rcb{h}", name=f"rcb{h}")
                nc.vector.tensor_copy(rcbh, rcf[h])
                rcb.append(rcbh)

            def mkc(fn, h):
                return lambda: fn(h)

            return [
                mkc(cl_dsb, 0), mkc(cl_dsb, 1),
                mkc(cl_rcf, 0), mkc(cl_rcf, 1),
                mkc(cl_rcb, 0), mkc(cl_rcb, 1),
            ]

        # ================= schedule =================
        NOP = []

        def F(*fns):
            return list(fns)

        def mk(fn, *args):
            return lambda: fn(*args)

        # ---- startup (image 0) ----
        stats_unit(0)
        gchain_unit(0)
        vt_init(0)
        for p in (0, 1):
            for which in (0, 1):
                for n in (0, 1):
                    qk_unit(0, p, which, n)

        # filler schedule per (image, pair): 8 slots, each a list.
        # Roughly one ~0.9us tensor unit per round so the PE never idles
        # (idle resets the clock p-state to half rate).  proj_a/proj_b
        # halves MUST occupy consecutive ps_f slots (open psum group).
        # tails sit at round 4: the pair's reciprocal chain (spread over
        # rounds 0-2 by emit_pair) finishes just before the bc matmuls.
        # dummy units pad tensor-light rounds so the PE clock never drops.
        sched = {}
        sched[(0, 0)] = [F(mk(vt_unit, 0, i)) for i in range(8)]
        sched[(0, 1)] = [
            F(mk(qk_unit, 0, 2, 0, 0)),
            F(mk(qk_unit, 0, 2, 0, 1)),
            F(mk(qk_unit, 0, 2, 1, 0)),
            F(mk(qk_unit, 0, 2, 1, 1)),
            F(mk(tail_unit, 0, 0)),
            F(mk(stats_unit, 1), mk(qk_unit, 0, 3, 0, 0)),
            F(mk(qk_unit, 0, 3, 0, 1)),
            F(mk(qk_unit, 0, 3, 1, 0)),
        ]
        sched[(0, 2)] = [
            F(mk(qk_unit, 0, 3, 1, 1)),
            F(mk(gchain_unit, 1), mk(dummy_unit, 1)),
            F(mk(vt_init, 1), mk(qk_unit, 1, 0, 0, 0)),
            F(mk(qk_unit, 1, 0, 0, 1)),
            F(mk(tail_unit, 0, 1)),
            F(mk(qk_unit, 1, 0, 1, 0)),
            F(mk(qk_unit, 1, 0, 1, 1)),
            F(mk(qk_unit, 1, 1, 0, 0)),
        ]
        sched[(0, 3)] = [
            F(mk(qk_unit, 1, 1, 0, 1)),
            F(mk(qk_unit, 1, 1, 1, 0)),
            F(mk(qk_unit, 1, 1, 1, 1)),
            F(mk(vt_unit, 1, 0)),
            F(mk(tail_unit, 0, 2)),
            F(mk(vt_unit, 1, 1)),
            F(mk(vt_unit, 1, 2)),
            F(mk(vt_unit, 1, 3)),
        ]
        sched[(1, 0)] = [
            F(mk(vt_unit, 1, 4)),
            F(mk(vt_unit, 1, 5)),
            F(mk(vt_unit, 1, 6)),
            F(mk(vt_unit, 1, 7)),
            F(mk(tail_unit, 0, 3)),
            F(mk(qk_unit, 1, 2, 0, 0)),
            F(mk(qk_unit, 1, 2, 0, 1)),
            F(mk(qk_unit, 1, 2, 1, 0)),
        ]
        sched[(1, 1)] = [
            F(mk(qk_unit, 1, 2, 1, 1)),
            F(mk(qk_unit, 1, 3, 0, 0)),
            F(mk(qk_unit, 1, 3, 0, 1)),
            F(mk(qk_unit, 1, 3, 1, 0)),
            F(mk(tail_unit, 1, 0)),
            F(mk(qk_unit, 1, 3, 1, 1)),
            F(mk(dummy_unit, 1), mk(proj_a, 0, 0, 0)),
            F(mk(proj_b, 0, 0, 0)),
        ]
        sched[(1, 2)] = [
            F(mk(dummy_unit, 1), mk(proj_a, 0, 0, 1)),
            F(mk(proj_b, 0, 0, 1)),
            F(mk(dummy_unit, 1), mk(proj_a, 0, 1, 0)),
            F(mk(proj_b, 0, 1, 0)),
            F(mk(tail_unit, 1, 1)),
            F(mk(dummy_unit, 1), mk(proj_a, 0, 1, 1)),
            F(mk(proj_b, 0, 1, 1)),
            F(mk(dummy_unit, 2)),
        ]
        sched[(1, 3)] = [
            F(mk(dummy_unit, 1), mk(proj_a, 0, 2, 0)),
            F(mk(proj_b, 0, 2, 0)),
            F(mk(dummy_unit, 1), mk(proj_a, 0, 2, 1)),
            F(mk(proj_b, 0, 2, 1)),
            F(mk(tail_unit, 1, 2)),
            F(mk(dummy_unit, 1), mk(proj_a, 0, 3, 0)),
            F(mk(proj_b, 0, 3, 0)),
            F(mk(proj_unit, 0, 3, 1)),
        ]

        dchain = []
        for b in range(BPC):
            for p in range(4):
                dchain = emit_pair(b, p, sched[(b, p)], dchain)

        # drain tail: keep the PE hot while the last pair's reciprocal
        # chain runs, then normalize + image-1 projection with psum pools
        # alternating so the units double-buffer
        for _ in range(5):
            dummy_unit(2)
        tail_unit(1, 3)
        for i, (m, n) in enumerate((m, n) for m in range(4) for n in range(2)):
            if i % 2 == 0:
                proj_unit(1, m, n)
            else:
                proj_unit(1, m, n, pool=ps_s, tag="s")

    nc.compile()
    return nc


def _get_nc():
    global _nc_cache
    if _nc_cache is None:
        _nc_cache = _build()
    return _nc_cache


def _prep_inputs(x, norm_w, norm_b, w_qkv, b_qkv, w_proj, b_proj):
    import ml_dtypes

    x = np.asarray(x, dtype=np.float32).reshape(B, C, L)
    w_qkv = np.asarray(w_qkv, dtype=np.float32)
    b_qkv = np.asarray(b_qkv, dtype=np.float32)
    w_proj = np.asarray(w_proj, dtype=np.float32)
    b_proj = np.asarray(b_proj, dtype=np.float32)
    norm_w = np.asarray(norm_w, dtype=np.float32)
    norm_b = np.asarray(norm_b, dtype=np.float32)

    # column-reordered transposed qkv weight: [C, 3C] with
    # q pair-major | k pair-major | v head-major, all contiguous
    wqkT = np.zeros((C, 3 * C), dtype=np.float32)
    wT = w_qkv.T  # [C, 3C] original row order (per head: q,k,v)
    for h in range(NH):
        base = 192 * h
        wqkT[:, 64 * h : 64 * (h + 1)] = wT[:, base : base + 64]
        wqkT[:, 512 + 64 * h : 512 + 64 * (h + 1)] = wT[:, base + 64 : base + 128]
        wqkT[:, 1024 + 64 * h : 1024 + 64 * (h + 1)] = wT[:, base + 128 : base + 192]
    wqkT = np.ascontiguousarray(wqkT.astype(ml_dtypes.bfloat16))
    wpT = np.ascontiguousarray(w_proj.T.astype(ml_dtypes.bfloat16))  # [C, C]

    # per-pair q/k biases: [pair, {q,k}, 128]
    bqk = np.zeros((4, 2, 128), dtype=np.float32)
    for p in range(4):
        for half, h in enumerate((2 * p, 2 * p + 1)):
            base = 192 * h
            bqk[p, 0, 64 * half : 64 * (half + 1)] = b_qkv[base : base + 64]
            bqk[p, 1, 64 * half : 64 * (half + 1)] = b_qkv[base + 64 : base + 128]
    del base

    # v bias folded into proj bias: b_eff = b_proj + w_proj @ bv
    bv = np.zeros((C,), dtype=np.float32)
    for h in range(NH):
        bv[64 * h : 64 * (h + 1)] = b_qkv[192 * h + 128 : 192 * h + 192]
    b_eff = (b_proj.astype(np.float64) + w_proj.astype(np.float64) @ bv).astype(
        np.float32
    )

    sel = np.zeros((128, 4 * NG), dtype=np.float32)
    esel = np.zeros((NG, 4 * 128), dtype=np.float32)
    for j in range(4):
        for c in range(128):
            sel[c, NG * j + 8 * j + c // GS] = 1.0 / GS
            esel[8 * j + c // GS, 128 * j + c] = 1.0

    cv = np.zeros((128, 20), dtype=np.float32)
    cv[:, 0:4] = bqk[:, 0, :].T
    cv[:, 4:8] = bqk[:, 1, :].T
    cv[:, 8:12] = b_eff.reshape(4, 128).T
    cv[:, 12:16] = norm_w.reshape(4, 128).T
    cv[:, 16:20] = norm_b.reshape(4, 128).T

    shared = {
        "wqkT": wqkT,
        "wpT": wpT,
        "cvec": cv,
        "sel": sel,
        "esel": esel,
    }
    in_maps = []
    for c in range(N_CORES):
        m = dict(shared)
        m["x2"] = np.ascontiguousarray(
            x[BPC * c : BPC * (c + 1)].astype(ml_dtypes.bfloat16)
        )
        in_maps.append(m)
    return in_maps


def _run(in_maps, trace=False):
    nc = _get_nc()
    return run_bass_kernel_spmd(
        nc, in_maps, core_ids=list(range(N_CORES)), trace=trace
    )


def kernel(x, norm_w, norm_b, w_qkv, b_qkv, w_proj, b_proj):
    in_maps = _prep_inputs(x, norm_w, norm_b, w_qkv, b_qkv, w_proj, b_proj)
    res = _run(in_maps)
    out = np.concatenate([r["out"] for r in res.results], axis=0)
    return out.astype(np.float32)
